# revision 1
# baseline (speedup 1.0000x reference)
"""HCNN (known-U) recurrence kernel for 8 Trainium2 NeuronCores.

Model (see reference): 80 sequential steps of
    state' = tanh(cat(post_state, u)) @ A            A: (2112, 2048) fp32
with teacher forcing post_state[:, :128] = y during the 64 past steps,
outputs = 64 past errors then 16 forecasts (first 128 state components).

Strategy
--------
Data-parallel over batch: 256 = 8 cores x 32. Each core runs the full
recurrence for its batch slice; no collectives.

Per-core per-step matmul x @ A with batch M=32 would waste 3/4 of the
128-wide PE array, so the A columns are split into 4 interleaved groups
and computed by 4 concurrent column-tiled matmuls (tile_position=(0,32j))
sharing the array. Data is fp16 (single pass): the teacher-forced
recurrence is strongly contractive, emulation shows end-to-end output
error ~1.5e-4 relative vs the fp32 reference.

Column interleave: state column s lives in col-group j=(s//32)%4 at free
offset 32*(s//128) + s%32. With that mapping the (128, 512) psum holding
state' (batch on partitions within each 32-group) turns into the next
step's stationary operand layout via a single DVE 32x32 block-transpose:
block (j, m') lands at partitions [32j:32j+32] of k-tile m' -- exactly
where matmul round m' reads it. ACT applies tanh (psum -> fp16 SBUF),
DVE transposes, PE consumes; y/u/init contributions are pre-tanh'ed and
pre-transposed on the host, so past-step rounds k=0 (y) and k=16 (u)
have no dependency on the transpose and hide its latency.
"""

import sys

for _p in ("/opt/trn_rl_repo", "/root/.axon_site/_ro/trn_rl_repo"):
    if _p not in sys.path:
        sys.path.insert(0, _p)

import numpy as np

N_STATE = 2048
N_U = 64
N_Y = 128
PAST = 64
FORE = 16
BATCH = 256
T = PAST + FORE          # 80 total steps; only 79 matmul steps needed
NSTEP = T - 1            # step t computes state_{t+1}; state_80 is unused
NK = 17                  # contraction tiles: 16 x 128 state + 1 x (64 u + 64 pad)
KDIM = NK * 128          # 2176 padded contraction size
N_CORES = 8
B = BATCH // N_CORES     # 32 per core


def _build_program():
    import concourse.bass as bass
    import concourse.tile as tile
    from concourse import mybir

    F32 = mybir.dt.float32
    F16 = mybir.dt.float16

    nc = bass.Bass("TRN2", target_bir_lowering=False, debug=False,
                   num_devices=N_CORES)

    A_ext = nc.declare_dram_parameter("A_re", [KDIM, 4, 512], F16, isOutput=False)
    ytanhT_ext = nc.declare_dram_parameter("ytanhT", [128, PAST * B], F16, isOutput=False)
    utanhT_ext = nc.declare_dram_parameter("utanhT", [128, NSTEP * B], F16, isOutput=False)
    ywrap_ext = nc.declare_dram_parameter("ywrap", [128, (PAST - 1) * B], F32, isOutput=False)
    initxT_ext = nc.declare_dram_parameter("initxT", [128, 512], F16, isOutput=False)
    out_ext = nc.declare_dram_parameter("outbuf", [128, NSTEP * B], F32, isOutput=True)

    with tile.TileContext(nc) as tc:
        with tc.tile_pool(name="const", bufs=1) as cpool, \
             tc.tile_pool(name="xbuf", bufs=2) as xpool, \
             tc.tile_pool(name="th", bufs=2) as thpool, \
             tc.tile_pool(name="psum", bufs=2, space="PSUM") as pspool:

            A_sb = cpool.tile([128, NK * 2048], F16, tag="A")
            for k in range(NK):
                nc.sync.dma_start(out=A_sb[:, 2048 * k:2048 * (k + 1)],
                                  in_=A_ext[128 * k:128 * (k + 1), :, :])
            ytanhT = cpool.tile([128, PAST * B], F16, tag="yt")
            nc.sync.dma_start(out=ytanhT[:], in_=ytanhT_ext[:])
            utanhT = cpool.tile([128, NSTEP * B], F16, tag="ut")
            nc.sync.dma_start(out=utanhT[:], in_=utanhT_ext[:])
            ywrap = cpool.tile([128, (PAST - 1) * B], F32, tag="yw")
            nc.sync.dma_start(out=ywrap[:], in_=ywrap_ext[:])
            outbuf = cpool.tile([128, NSTEP * B], F32, tag="ob")

            xlo = xpool.tile([128, 256], F16, tag="xlo")
            xhi = xpool.tile([128, 256], F16, tag="xhi")
            nc.sync.dma_start(out=xlo[:], in_=initxT_ext[:, 0:256])
            nc.sync.dma_start(out=xhi[:], in_=initxT_ext[:, 256:512])

            def lhs_for(t, k, lo, hi):
                if k == 0:
                    if t < PAST:
                        return ytanhT[:, B * t:B * (t + 1)]
                    return lo[:, 0:32]
                if k == 16:
                    return utanhT[:, B * t:B * (t + 1)]
                if k < 8:
                    return lo[:, 32 * k:32 * (k + 1)]
                return hi[:, 32 * (k - 8):32 * (k - 7)]

            for t in range(NSTEP):
                ps = pspool.tile([128, 512], F32, tag="ps")
                # k emission order: y and u tiles first (no transpose dep),
                # then the state tiles as the transposes complete.
                korder = [0, 16] + list(range(1, 16))
                for idx, k in enumerate(korder):
                    lhsT = lhs_for(t, k, xlo, xhi)
                    start = idx == 0
                    stop = idx == len(korder) - 1
                    for j in range(4):
                        nc.tensor.matmul(
                            ps[32 * j:32 * (j + 1), :],
                            lhsT,
                            A_sb[:, 2048 * k + 512 * j:2048 * k + 512 * (j + 1)],
                            start=start, stop=stop,
                            tile_position=(0, 32 * j),
                        )

                # output slot t+1 from this psum (expectation = cols 0:128 of
                # state', living in psum[:, 0:32] across all partition groups)
                if t + 1 < PAST:
                    nc.vector.tensor_sub(outbuf[:, B * t:B * (t + 1)],
                                         ps[:, 0:32],
                                         ywrap[:, B * t:B * (t + 1)])
                else:
                    nc.vector.tensor_copy(outbuf[:, B * t:B * (t + 1)],
                                          ps[:, 0:32])

                if t < NSTEP - 1:
                    th_lo = thpool.tile([128, 256], F16, tag="thlo")
                    nc.scalar.activation(th_lo[:], ps[:, 0:256],
                                         mybir.ActivationFunctionType.Tanh)
                    nlo = xpool.tile([128, 256], F16, tag="xlo")
                    nc.vector.transpose(nlo[:], th_lo[:])
                    th_hi = thpool.tile([128, 256], F16, tag="thhi")
                    nc.scalar.activation(th_hi[:], ps[:, 256:512],
                                         mybir.ActivationFunctionType.Tanh)
                    nhi = xpool.tile([128, 256], F16, tag="xhi")
                    nc.vector.transpose(nhi[:], th_hi[:])
                    xlo, xhi = nlo, nhi

            nc.sync.dma_start(out=out_ext[:], in_=outbuf[:])

    _split_multi_waits(nc)
    return nc


def _split_multi_waits(nc):
    """This walrus build accepts at most one sem wait per instruction; Tile
    sometimes emits more. Hoist extras onto nops inserted just before the
    instruction in the same engine stream."""
    from concourse import mybir

    n = 0
    for f in nc.m.functions:
        for b in f.blocks:
            insts = b.instructions
            out = []
            changed = False
            for ins in insts:
                si = ins.sync_info
                if si is not None and len(si.on_wait) > 1:
                    waits = list(si.on_wait)
                    for w in waits[:-1]:
                        n += 1
                        out.append(mybir.InstNoOp(
                            name=f"I-waitsplit-{n}",
                            engine=ins.engine,
                            ins=[], outs=[],
                            bass_nofuse=True,
                            sync_info=mybir.SyncInfo(on_wait=[w], on_update=[]),
                        ))
                    ins.sync_info = mybir.SyncInfo(
                        on_wait=[waits[-1]], on_update=list(si.on_update))
                    changed = True
                out.append(ins)
            if changed:
                b.instructions = out


def _host_inputs(U, Y, A, init_state):
    """Build the per-core input maps (all pre-tanh / pre-transpose work)."""
    A = np.asarray(A, np.float32)
    U = np.asarray(U, np.float32)
    Y = np.asarray(Y, np.float32)
    init_state = np.asarray(init_state, np.float32)

    A_pad = np.zeros((KDIM, N_STATE), np.float16)
    A_pad[:N_STATE + N_U] = A.astype(np.float16)
    # column interleave: col s -> (j=(s//32)%4, free 32*(s//128)+s%32)
    A_re = np.ascontiguousarray(
        A_pad.reshape(KDIM, 16, 4, 32).transpose(0, 2, 1, 3).reshape(KDIM, 4, 512))

    init_tanh = np.tanh(init_state[0]).astype(np.float16)          # (2048,)
    initxT = np.ascontiguousarray(
        np.broadcast_to(init_tanh.reshape(16, 128).T[:, None, :].transpose(0, 2, 1),
                        (128, 16, 32)).reshape(128, 512))

    ytanh = np.tanh(Y).astype(np.float16)                          # (64, 256, 128)
    utanh = np.tanh(U[:NSTEP]).astype(np.float16)                  # (79, 256, 64)

    in_maps = []
    for c in range(N_CORES):
        b0 = c * B
        yt = np.ascontiguousarray(
            ytanh[:, b0:b0 + B, :].transpose(0, 2, 1)              # (64, 128, 32)
            .transpose(1, 0, 2).reshape(128, PAST * B))
        ut = np.zeros((128, NSTEP * B), np.float16)
        ut[:N_U] = (utanh[:, b0:b0 + B, :].transpose(0, 2, 1)      # (79, 64, 32)
                    .transpose(1, 0, 2).reshape(N_U, NSTEP * B))
        # ywrap slot s (=1..63) at cols 32*(s-1): rows 32j+b = Y[s, b0+b, 32j+cc]
        yw = (Y[1:PAST, b0:b0 + B, :].reshape(PAST - 1, B, 4, 32)
              .transpose(0, 2, 1, 3)                               # (63, 4, 32b, 32cc)
              .reshape(PAST - 1, 128, 32)
              .transpose(1, 0, 2).reshape(128, (PAST - 1) * B))
        in_maps.append({
            "A_re": A_re,
            "ytanhT": yt,
            "utanhT": np.ascontiguousarray(ut),
            "ywrap": np.ascontiguousarray(yw.astype(np.float32)),
            "initxT": initxT,
        })
    return in_maps


def kernel(U, Y, A, init_state):
    from concourse.bass_utils import run_bass_kernel_spmd

    nc = _build_program()
    in_maps = _host_inputs(U, Y, A, init_state)
    res = run_bass_kernel_spmd(nc, in_maps, list(range(N_CORES)))

    out = np.empty((T, BATCH, N_Y), np.float32)
    # slot 0: err for t=0 is pure host math (state_0 = broadcast init_state)
    out[0] = np.asarray(init_state, np.float32)[0, :N_Y][None, :] - np.asarray(Y, np.float32)[0]
    for c in range(N_CORES):
        b0 = c * B
        ob = res.results[c]["outbuf"]                              # (128, 79*32)
        # [32j+b, 32t+cc] = out[t+1, b0+b, 32j+cc]
        ob4 = ob.reshape(4, 32, NSTEP, 32)                         # (j, b, t, cc)
        out[1:, b0:b0 + B, :] = ob4.transpose(2, 1, 0, 3).reshape(NSTEP, B, N_Y)
    return out


if __name__ == "__main__":
    rng = np.random.default_rng(0)
    U = rng.standard_normal((T, BATCH, N_U)).astype(np.float32)
    Y = rng.standard_normal((PAST, BATCH, N_Y)).astype(np.float32)
    A = (rng.standard_normal((N_STATE + N_U, N_STATE)) * 0.02).astype(np.float32)
    init = rng.standard_normal((1, N_STATE)).astype(np.float32)
    o = kernel(U=U, Y=Y, A=A, init_state=init)
    print("kernel out:", o.shape, o.dtype)

